# revision 23
# baseline (speedup 1.0000x reference)
"""MoE downsample kernel for 8 TRN2 NeuronCores — top-2-only compute.

The reference computes all 4 experts densely but only the host-computable
top-2 gate survives to the output, so the device computes just the 32
selected (sample, expert) convs (seed-0 demand: 1112 taps vs 2624 dense).

Work distribution keeps one compiled SPMD program with perfect balance:
every core computes output rows [16c, 16c+16) of EVERY sample (16
segments/core). Within a segment the input window is column-split across
the two PE row halves (half r covers output cols 64r..64r+63), and the
two PSUM col halves swap (expert, row-block) assignments so each of the
four 64x64 quadrant queues does exactly t_e1 + t_e2 tap-matmuls per
segment. Chunk-task = 8 rows x 64 cols = 512 px = one PSUM bank.
BN + conv-bias + GELU fuse into the ScalarE PSUM eviction. Gating and
final top-2 scale/assembly run on host.

DMA is the co-bottleneck (~47 MB moved against a ~208 GB/s packet-rate
ceiling), so: windows are staged CONTIGUOUSLY by the host and DMAed flat
(10-20 KB packets instead of ~300 B window rows), outputs are stored
bf16 (halves output bytes) on the scalar HWDGE ring while windows ride
the sync HWDGE ring, window loads are emitted two segments ahead, PSUM
banks are shared pairwise across the PSUM col halves for slack, and
light segments go first/last to shorten the DMA-bound head and the
eviction tail.
"""

import numpy as np
import ml_dtypes

KS = [3, 5, 7, 9]
DS = [1, 2, 3, 4]
PADS = [1, 4, 9, 16]       # d*(k-1)//2
TAPN = [9, 25, 49, 81]
BN_EPS = 1e-5
B, CIN, H, W = 16, 64, 256, 256
CE = 64
HO = WO = 128
NCORES = 8
PAD = 16                   # left/top pad in the padded image
HP = WP = PAD + 256 + 15   # 287
NSEG = B                   # one segment per sample per core
RB = 8                     # output rows per chunk-task block
PF = 3                     # window prefetch distance (segments)
NWARM = 48                 # dummy matmuls to pre-warm the PE HAM clock


def _blk_shape(e):
    """Per-expert window block: decimated (stride-2 gather, unit-stride
    taps) for even dilation, full-res otherwise. Returns (dec, p, R, W)."""
    p = PADS[e]
    if DS[e] % 2 == 0:
        return True, p, 16 + p, 64 + p
    return False, p, 31 + 2 * p, 127 + 2 * p


def _win_layout(pair):
    """[(e, dec, p, R, W, off)] block list + total flat length."""
    off, blocks = 0, []
    for e in pair:
        dec, p, R, Wd = _blk_shape(e)
        blocks.append((e, dec, p, R, Wd, off))
        off += R * Wd
    return blocks, off


WIN_FLAT = max(_win_layout((a, b))[1]
               for a in range(4) for b in range(4) if a != b)

_COMPILED = {}


def _tap_offsets(e):
    """Yield (local_slot, row_off, col_off) raw offsets for expert e."""
    k, d = KS[e], DS[e]
    pad = d * (k - 1) // 2
    for u in range(k):
        for v in range(k):
            yield u * k + v, d * u - pad, d * v - pad


def _seg_order(pairs):
    """Spread the lightest segments evenly (incl. first and last slots):
    light segments have the smallest windows (fast start) and shortest
    eviction tails; heavies between them give DMA prefetch headroom."""
    taps = [TAPN[a] + TAPN[b] for a, b in pairs]
    by_weight = sorted(range(NSEG), key=lambda s: (taps[s], s))
    nl = (NSEG + 2) // 3
    light_pos = sorted({round(i * (NSEG - 1) / max(nl - 1, 1))
                        for i in range(nl)})
    order = [None] * NSEG
    for pos, s in zip(light_pos, by_weight[:len(light_pos)]):
        order[pos] = s
    rest = iter(sorted(by_weight[len(light_pos):],
                       key=lambda s: (-taps[s], s)))
    for i in range(NSEG):
        if order[i] is None:
            order[i] = next(rest)
    return order


def _build_program(pairs, order, compile=True):
    import concourse.bass as bass  # noqa: F401
    import concourse.mybir as mybir
    import concourse.tile as tile
    from concourse import bacc
    from contextlib import ExitStack

    dt = mybir.dt
    nc = bacc.Bacc("TRN2", target_bir_lowering=False, debug=False,
                   num_devices=NCORES)
    xwin = nc.dram_tensor("xwin", [NSEG, 2, CIN, WIN_FLAT], dt.bfloat16,
                          kind="ExternalInput")
    wt = nc.dram_tensor("wt", [CIN, sum(TAPN), CE], dt.bfloat16,
                        kind="ExternalInput")
    bnp = nc.dram_tensor("bnp", [CE, 4, 2], dt.float32, kind="ExternalInput")
    # [seg, rank, row-block, col-half, ch, row, col]
    out = nc.dram_tensor("out", [NSEG, 2, 2, 2, CE, RB, 64], dt.bfloat16,
                         kind="ExternalOutput")
    slot_base = np.cumsum([0] + [k * k for k in KS]).tolist()

    with tile.TileContext(nc) as tc:
        with ExitStack() as ctx:
            consts = ctx.enter_context(tc.tile_pool(name="consts", bufs=1))
            win_pool = ctx.enter_context(tc.tile_pool(name="win", bufs=5))
            stage_pool = ctx.enter_context(tc.tile_pool(name="st", bufs=8))
            psum_pool = ctx.enter_context(
                tc.tile_pool(name="ps", bufs=8, space="PSUM"))

            # per-expert weight tiles so early matmuls only wait on the
            # weights they actually use; first segment's experts load first
            wts = {}
            bntile = consts.tile([128, 4, 2], dt.float32)
            e_order = list(pairs[order[0]])
            e_order += [e for e in range(4) if e not in e_order]

            def load_weights(e):
                t = consts.tile([128, KS[e] * KS[e], CE], dt.bfloat16,
                                tag=f"wt{e}", name=f"wt{e}")
                for half in range(2):
                    p0 = half * 64
                    nc.sync.dma_start(
                        out=t[p0:p0 + 64, :, :],
                        in_=wt[:, slot_base[e]:slot_base[e] + KS[e] * KS[e],
                               :])
                wts[e] = t

            win_tiles = {}

            def issue_window(j):
                seg = order[j]
                _, flat_len = _win_layout(pairs[seg])
                win = win_pool.tile([128, WIN_FLAT], dt.bfloat16, name="win")
                for r in range(2):
                    nc.sync.dma_start(
                        out=win[64 * r:64 * r + 64, 0:flat_len],
                        in_=xwin[seg, r, :, 0:flat_len])
                win_tiles[j] = win

            load_weights(e_order[0])
            load_weights(e_order[1])
            for half in range(2):
                p0 = half * 64
                nc.sync.dma_start(out=bntile[p0:p0 + 64, :, :],
                                  in_=bnp.ap())
            issue_window(0)
            for e in e_order[2:]:
                load_weights(e)
            for j in range(1, 1 + PF):
                issue_window(j)

            # keep the PE HAM clock warm through the startup DMA wait:
            # zero matmuls into the first segment's bank; the real first
            # tap has start=True so the garbage is overwritten
            scratch = consts.tile([128, 512], dt.bfloat16)
            nc.vector.memset(scratch, 0.0)

            for j in range(NSEG):
                seg = order[j]
                e1, e2 = pairs[seg]
                if j + PF + 1 < NSEG:
                    issue_window(j + PF + 1)
                win = win_tiles.pop(j)
                # per-expert [128, R, W] views of the flat-packed blocks
                views = {}
                for (e, dec, pe, R, Wd, off) in _win_layout((e1, e2))[0]:
                    views[e] = (win[:, off:off + R * Wd].rearrange(
                        "p (r w) -> p r w", w=Wd), dec, pe)
                # PSUM banks shared across the two col-halves per
                # (phase, row-half): 4 live banks per segment
                ps_seg = {(ph, r): psum_pool.tile([128, 512], dt.float32,
                                                  name="psb")
                          for ph in range(2) for r in range(2)}
                if j == 0:
                    for _ in range(NWARM):
                        nc.tensor.matmul(ps_seg[(0, 0)][0:64, :],
                                         scratch[0:64, 0:64],
                                         scratch[0:64, :], start=True,
                                         stop=True, tile_position=(0, 0))

                def task_events(e, blk, r, c, ph, views=views, seg=seg,
                                e1=e1, ps_seg=ps_seg):
                    ps = ps_seg[(ph, r)]
                    q0 = c * 64
                    p0 = r * 64
                    taps = list(_tap_offsets(e))
                    wte = wts[e]
                    view, dec, pe = views[e]
                    for t, (slot, ro, co) in enumerate(taps):
                        if dec:
                            r_lo = 8 * blk + ro // 2 + pe // 2
                            c_lo = co // 2 + pe // 2
                            rhs = view[p0:p0 + 64, r_lo:r_lo + 8,
                                       c_lo:c_lo + 64]
                        else:
                            r_lo = 16 * blk + ro + pe
                            c_lo = co + pe
                            rhs = view[p0:p0 + 64, r_lo:r_lo + 15:2,
                                       c_lo:c_lo + 127:2]
                        lhsT = wte[p0:p0 + 64, slot, :]
                        psv = ps[q0:q0 + 64, :]
                        first = t == 0
                        last = t == len(taps) - 1

                        def mm(rhs=rhs, lhsT=lhsT, psv=psv, first=first,
                               last=last, p0=p0, q0=q0):
                            nc.tensor.matmul(psv, lhsT, rhs, start=first,
                                             stop=last,
                                             tile_position=(p0, q0))
                        yield mm

                    def evict(ps=ps, e=e, blk=blk, r=r, q0=q0, seg=seg,
                              e1=e1):
                        st = stage_pool.tile([128, RB, 64], dt.bfloat16,
                                             name="st")
                        nc.scalar.activation(
                            st[q0:q0 + 64, :, :],
                            ps[q0:q0 + 64, :].rearrange(
                                "p (a b) -> p a b", a=RB),
                            mybir.ActivationFunctionType.Gelu,
                            scale=bntile[q0:q0 + 64, e, 0:1],
                            bias=bntile[q0:q0 + 64, e, 1:2])
                        rank = 0 if e == e1 else 1
                        nc.scalar.dma_start(
                            out=out[seg, rank, blk, r, :, :, :],
                            in_=st[q0:q0 + 64, :, :])
                    yield evict

                def qgen(r, c, e1=e1, e2=e2):
                    ea, eb = (e1, e2) if c == 0 else (e2, e1)
                    yield from task_events(ea, 0, r, c, 0)
                    yield from task_events(eb, 1, r, c, 1)

                queues = [qgen(r, c) for r in range(2) for c in range(2)]
                live = list(queues)
                while live:
                    nxt = []
                    for q in live:
                        ev = next(q, None)
                        if ev is None:
                            continue
                        ev()
                        nxt.append(q)
                    live = nxt

    if compile:
        nc.compile()
    return nc


def _get_program(pairs, order):
    key = (pairs, tuple(order))
    if key not in _COMPILED:
        _COMPILED[key] = _build_program(pairs, order)
    return _COMPILED[key]


def _host_gate(x, gate_w, gate_b):
    """Replicate reference gating in numpy (f64 pooling for robustness)."""
    pooled = x.astype(np.float64).mean(axis=(2, 3)).astype(np.float32)
    logits = pooled @ gate_w.T.astype(np.float32) + gate_b
    z = logits - logits.max(axis=1, keepdims=True)
    ez = np.exp(z.astype(np.float32))
    gates = ez / ez.sum(axis=1, keepdims=True)
    idx = np.argsort(-gates, axis=1, kind="stable")[:, :2]
    wsel = np.take_along_axis(gates, idx, axis=1)
    wsel = wsel / (wsel.sum(axis=1, keepdims=True) + 1e-8)
    return idx, wsel.astype(np.float32)


def _prep_inputs(x, ws, bs, bn_scale, bn_bias, bn_mean, bn_var, pairs):
    bf16 = ml_dtypes.bfloat16
    slot_base = np.cumsum([0] + [k * k for k in KS]).tolist()
    xpad = np.zeros((B, CIN, HP, WP), dtype=bf16)
    xpad[:, :, PAD:PAD + H, PAD:PAD + W] = x.astype(bf16)

    # contiguous flat per-expert window blocks per (core, sample, half):
    # decimated (stride-2 gather) for even-dilation experts
    xwin_cores = []
    for c in range(NCORES):
        xw = np.zeros((NSEG, 2, CIN, WIN_FLAT), dtype=bf16)
        for s in range(NSEG):
            blocks, _ = _win_layout(pairs[s])
            for r in range(2):
                for (e, dec, p, R, Wd, off) in blocks:
                    r0 = 32 * c + 16 - p
                    c0 = 128 * r + 16 - p
                    if dec:
                        blk = xpad[s, :, r0:r0 + 2 * R:2, c0:c0 + 2 * Wd:2]
                    else:
                        blk = xpad[s, :, r0:r0 + R, c0:c0 + Wd]
                    xw[s, r, :, off:off + R * Wd] = blk.reshape(CIN, R * Wd)
        xwin_cores.append(xw)

    wt = np.empty((CIN, sum(TAPN), CE), dtype=bf16)
    for e in range(4):
        k = KS[e]
        w = ws[e].astype(np.float32)  # [CE, CIN, k, k]
        wt[:, slot_base[e]:slot_base[e] + k * k, :] = (
            w.transpose(1, 2, 3, 0).reshape(CIN, k * k, CE).astype(bf16))

    inv = (bn_scale / np.sqrt(bn_var + BN_EPS)).astype(np.float32)
    shift = (np.stack(bs) * inv + bn_bias - bn_mean * inv).astype(np.float32)
    bnp = np.stack([inv, shift], axis=1)  # [4, 2, CE]
    bnp = np.ascontiguousarray(bnp.transpose(2, 0, 1))  # [CE, 4, 2]
    return xwin_cores, wt, bnp


def run(inputs, trace=False):
    from concourse import bass_utils

    x = np.asarray(inputs["x"], dtype=np.float32)
    ws = [np.asarray(inputs[f"w{i}"], dtype=np.float32) for i in range(4)]
    bs = [np.asarray(inputs[f"b{i}"], dtype=np.float32) for i in range(4)]
    bn_scale = np.asarray(inputs["bn_scale"], dtype=np.float32)
    bn_bias = np.asarray(inputs["bn_bias"], dtype=np.float32)
    bn_mean = np.asarray(inputs["bn_mean"], dtype=np.float32)
    bn_var = np.asarray(inputs["bn_var"], dtype=np.float32)
    gate_w = np.asarray(inputs["gate_w"], dtype=np.float32)
    gate_b = np.asarray(inputs["gate_b"], dtype=np.float32)

    idx, wsel = _host_gate(x, gate_w, gate_b)
    pairs = tuple((int(idx[s, 0]), int(idx[s, 1])) for s in range(B))
    order = _seg_order(pairs)

    nc = _get_program(pairs, order)
    xwin_cores, wt, bnp = _prep_inputs(x, ws, bs, bn_scale, bn_bias,
                                       bn_mean, bn_var, pairs)
    in_maps = [{"xwin": xwin_cores[c], "wt": wt, "bnp": bnp}
               for c in range(NCORES)]
    res = bass_utils.run_bass_kernel_spmd(
        nc, in_maps, core_ids=list(range(NCORES)), trace=trace)

    # assemble: core c holds rows [16c, 16c+16) of every (sample, rank)
    E = np.empty((B, 2, CE, HO, WO), dtype=np.float32)
    for c in range(NCORES):
        o = res.results[c]["out"]  # [seg, rank, blk, rhalf, ch, row, col]
        t = (o.astype(np.float32)
             .transpose(0, 1, 4, 2, 5, 3, 6).reshape(NSEG, 2, CE, 16, 128))
        E[:, :, :, 16 * c:16 * c + 16, :] = t
    outf = np.empty((B, 2 * CE, HO, WO), dtype=np.float32)
    for s in range(B):
        outf[s, :CE] = E[s, 0] * wsel[s, 0]
        outf[s, CE:] = E[s, 1] * wsel[s, 1]
    return outf, res


def kernel(**inputs):
    outf, _ = run(inputs, trace=False)
    return outf


# revision 24
# speedup vs baseline: 1.0006x; 1.0006x over previous
"""MoE downsample kernel for 8 TRN2 NeuronCores — top-2-only compute.

The reference computes all 4 experts densely but only the host-computable
top-2 gate survives to the output, so the device computes just the 32
selected (sample, expert) convs (seed-0 demand: 1112 taps vs 2624 dense).

Work distribution keeps one compiled SPMD program with perfect balance:
every core computes output rows [16c, 16c+16) of EVERY sample (16
segments/core). Within a segment the input window is column-split across
the two PE row halves (half r covers output cols 64r..64r+63), and the
two PSUM col halves swap (expert, row-block) assignments so each of the
four 64x64 quadrant queues does exactly t_e1 + t_e2 tap-matmuls per
segment. Chunk-task = 8 rows x 64 cols = 512 px = one PSUM bank.
BN + conv-bias + GELU fuse into the ScalarE PSUM eviction. Gating and
final top-2 scale/assembly run on host.

DMA is the co-bottleneck (~47 MB moved against a ~208 GB/s packet-rate
ceiling), so: windows are staged CONTIGUOUSLY by the host and DMAed flat
(10-20 KB packets instead of ~300 B window rows), outputs are stored
bf16 (halves output bytes) on the scalar HWDGE ring while windows ride
the sync HWDGE ring, window loads are emitted two segments ahead, PSUM
banks are shared pairwise across the PSUM col halves for slack, and
light segments go first/last to shorten the DMA-bound head and the
eviction tail.
"""

import numpy as np
import ml_dtypes

KS = [3, 5, 7, 9]
DS = [1, 2, 3, 4]
PADS = [1, 4, 9, 16]       # d*(k-1)//2
TAPN = [9, 25, 49, 81]
BN_EPS = 1e-5
B, CIN, H, W = 16, 64, 256, 256
CE = 64
HO = WO = 128
NCORES = 8
PAD = 16                   # left/top pad in the padded image
HP = WP = PAD + 256 + 15   # 287
NSEG = B                   # one segment per sample per core
RB = 8                     # output rows per chunk-task block
PF = 2                     # window prefetch distance (segments)
NWARM = 48                 # dummy matmuls to pre-warm the PE HAM clock


def _blk_shape(e):
    """Per-expert window block: decimated (stride-2 gather, unit-stride
    taps) for even dilation, full-res otherwise. Returns (dec, p, R, W)."""
    p = PADS[e]
    if DS[e] % 2 == 0:
        return True, p, 16 + p, 64 + p
    return False, p, 31 + 2 * p, 127 + 2 * p


def _win_layout(pair):
    """[(e, dec, p, R, W, off)] block list + total flat length."""
    off, blocks = 0, []
    for e in pair:
        dec, p, R, Wd = _blk_shape(e)
        blocks.append((e, dec, p, R, Wd, off))
        off += R * Wd
    return blocks, off


WIN_FLAT = max(_win_layout((a, b))[1]
               for a in range(4) for b in range(4) if a != b)

_COMPILED = {}


def _tap_offsets(e):
    """Yield (local_slot, row_off, col_off) raw offsets for expert e."""
    k, d = KS[e], DS[e]
    pad = d * (k - 1) // 2
    for u in range(k):
        for v in range(k):
            yield u * k + v, d * u - pad, d * v - pad


def _seg_order(pairs):
    """Spread the lightest segments evenly (incl. first and last slots):
    light segments have the smallest windows (fast start) and shortest
    eviction tails; heavies between them give DMA prefetch headroom."""
    taps = [TAPN[a] + TAPN[b] for a, b in pairs]
    by_weight = sorted(range(NSEG), key=lambda s: (taps[s], s))
    nl = (NSEG + 2) // 3
    light_pos = sorted({round(i * (NSEG - 1) / max(nl - 1, 1))
                        for i in range(nl)})
    order = [None] * NSEG
    for pos, s in zip(light_pos, by_weight[:len(light_pos)]):
        order[pos] = s
    rest = iter(sorted(by_weight[len(light_pos):],
                       key=lambda s: (-taps[s], s)))
    for i in range(NSEG):
        if order[i] is None:
            order[i] = next(rest)
    return order


def _build_program(pairs, order, compile=True):
    import concourse.bass as bass  # noqa: F401
    import concourse.mybir as mybir
    import concourse.tile as tile
    from concourse import bacc
    from contextlib import ExitStack

    dt = mybir.dt
    nc = bacc.Bacc("TRN2", target_bir_lowering=False, debug=False,
                   num_devices=NCORES)
    xwin = nc.dram_tensor("xwin", [NSEG, 2, CIN, WIN_FLAT], dt.bfloat16,
                          kind="ExternalInput")
    wt = nc.dram_tensor("wt", [CIN, sum(TAPN), CE], dt.bfloat16,
                        kind="ExternalInput")
    bnp = nc.dram_tensor("bnp", [CE, 4, 2], dt.float32, kind="ExternalInput")
    # [seg, rank, row-block, col-half, ch, row, col]
    out = nc.dram_tensor("out", [NSEG, 2, 2, 2, CE, RB, 64], dt.bfloat16,
                         kind="ExternalOutput")
    slot_base = np.cumsum([0] + [k * k for k in KS]).tolist()

    with tile.TileContext(nc) as tc:
        with ExitStack() as ctx:
            consts = ctx.enter_context(tc.tile_pool(name="consts", bufs=1))
            win_pool = ctx.enter_context(tc.tile_pool(name="win", bufs=4))
            stage_pool = ctx.enter_context(tc.tile_pool(name="st", bufs=8))
            psum_pool = ctx.enter_context(
                tc.tile_pool(name="ps", bufs=8, space="PSUM"))

            # per-expert weight tiles so early matmuls only wait on the
            # weights they actually use; first segment's experts load first
            wts = {}
            bntile = consts.tile([128, 4, 2], dt.float32)
            e_order = list(pairs[order[0]])
            e_order += [e for e in range(4) if e not in e_order]

            def load_weights(e):
                t = consts.tile([128, KS[e] * KS[e], CE], dt.bfloat16,
                                tag=f"wt{e}", name=f"wt{e}")
                for half in range(2):
                    p0 = half * 64
                    nc.sync.dma_start(
                        out=t[p0:p0 + 64, :, :],
                        in_=wt[:, slot_base[e]:slot_base[e] + KS[e] * KS[e],
                               :])
                wts[e] = t

            win_tiles = {}

            def issue_window(j):
                seg = order[j]
                _, flat_len = _win_layout(pairs[seg])
                win = win_pool.tile([128, WIN_FLAT], dt.bfloat16, name="win")
                for r in range(2):
                    nc.sync.dma_start(
                        out=win[64 * r:64 * r + 64, 0:flat_len],
                        in_=xwin[seg, r, :, 0:flat_len])
                win_tiles[j] = win

            load_weights(e_order[0])
            load_weights(e_order[1])
            for half in range(2):
                p0 = half * 64
                nc.sync.dma_start(out=bntile[p0:p0 + 64, :, :],
                                  in_=bnp.ap())
            issue_window(0)
            for e in e_order[2:]:
                load_weights(e)
            for j in range(1, 1 + PF):
                issue_window(j)

            # keep the PE HAM clock warm through the startup DMA wait:
            # zero matmuls into the first segment's bank; the real first
            # tap has start=True so the garbage is overwritten
            scratch = consts.tile([128, 512], dt.bfloat16)
            nc.vector.memset(scratch, 0.0)

            for j in range(NSEG):
                seg = order[j]
                e1, e2 = pairs[seg]
                if j + PF + 1 < NSEG:
                    issue_window(j + PF + 1)
                win = win_tiles.pop(j)
                # per-expert [128, R, W] views of the flat-packed blocks
                views = {}
                for (e, dec, pe, R, Wd, off) in _win_layout((e1, e2))[0]:
                    views[e] = (win[:, off:off + R * Wd].rearrange(
                        "p (r w) -> p r w", w=Wd), dec, pe)
                # PSUM banks shared across the two col-halves per
                # (phase, row-half): 4 live banks per segment
                ps_seg = {(ph, r): psum_pool.tile([128, 512], dt.float32,
                                                  name="psb")
                          for ph in range(2) for r in range(2)}
                if j == 0:
                    for _ in range(NWARM):
                        nc.tensor.matmul(ps_seg[(0, 0)][0:64, :],
                                         scratch[0:64, 0:64],
                                         scratch[0:64, :], start=True,
                                         stop=True, tile_position=(0, 0))

                def task_events(e, blk, r, c, ph, views=views, seg=seg,
                                e1=e1, ps_seg=ps_seg):
                    ps = ps_seg[(ph, r)]
                    q0 = c * 64
                    p0 = r * 64
                    taps = list(_tap_offsets(e))
                    wte = wts[e]
                    view, dec, pe = views[e]
                    for t, (slot, ro, co) in enumerate(taps):
                        if dec:
                            r_lo = 8 * blk + ro // 2 + pe // 2
                            c_lo = co // 2 + pe // 2
                            rhs = view[p0:p0 + 64, r_lo:r_lo + 8,
                                       c_lo:c_lo + 64]
                        else:
                            r_lo = 16 * blk + ro + pe
                            c_lo = co + pe
                            rhs = view[p0:p0 + 64, r_lo:r_lo + 15:2,
                                       c_lo:c_lo + 127:2]
                        lhsT = wte[p0:p0 + 64, slot, :]
                        psv = ps[q0:q0 + 64, :]
                        first = t == 0
                        last = t == len(taps) - 1

                        def mm(rhs=rhs, lhsT=lhsT, psv=psv, first=first,
                               last=last, p0=p0, q0=q0):
                            nc.tensor.matmul(psv, lhsT, rhs, start=first,
                                             stop=last,
                                             tile_position=(p0, q0))
                        yield mm

                    def evict(ps=ps, e=e, blk=blk, r=r, q0=q0, seg=seg,
                              e1=e1):
                        st = stage_pool.tile([128, RB, 64], dt.bfloat16,
                                             name="st")
                        nc.scalar.activation(
                            st[q0:q0 + 64, :, :],
                            ps[q0:q0 + 64, :].rearrange(
                                "p (a b) -> p a b", a=RB),
                            mybir.ActivationFunctionType.Gelu,
                            scale=bntile[q0:q0 + 64, e, 0:1],
                            bias=bntile[q0:q0 + 64, e, 1:2])
                        rank = 0 if e == e1 else 1
                        nc.scalar.dma_start(
                            out=out[seg, rank, blk, r, :, :, :],
                            in_=st[q0:q0 + 64, :, :])
                    yield evict

                def qgen(r, c, e1=e1, e2=e2):
                    ea, eb = (e1, e2) if c == 0 else (e2, e1)
                    yield from task_events(ea, 0, r, c, 0)
                    yield from task_events(eb, 1, r, c, 1)

                queues = [qgen(r, c) for r in range(2) for c in range(2)]
                live = list(queues)
                while live:
                    nxt = []
                    for q in live:
                        ev = next(q, None)
                        if ev is None:
                            continue
                        ev()
                        nxt.append(q)
                    live = nxt

    if compile:
        nc.compile()
    return nc


def _get_program(pairs, order):
    key = (pairs, tuple(order))
    if key not in _COMPILED:
        _COMPILED[key] = _build_program(pairs, order)
    return _COMPILED[key]


def _host_gate(x, gate_w, gate_b):
    """Replicate reference gating in numpy (f64 pooling for robustness)."""
    pooled = x.astype(np.float64).mean(axis=(2, 3)).astype(np.float32)
    logits = pooled @ gate_w.T.astype(np.float32) + gate_b
    z = logits - logits.max(axis=1, keepdims=True)
    ez = np.exp(z.astype(np.float32))
    gates = ez / ez.sum(axis=1, keepdims=True)
    idx = np.argsort(-gates, axis=1, kind="stable")[:, :2]
    wsel = np.take_along_axis(gates, idx, axis=1)
    wsel = wsel / (wsel.sum(axis=1, keepdims=True) + 1e-8)
    return idx, wsel.astype(np.float32)


def _prep_inputs(x, ws, bs, bn_scale, bn_bias, bn_mean, bn_var, pairs):
    bf16 = ml_dtypes.bfloat16
    slot_base = np.cumsum([0] + [k * k for k in KS]).tolist()
    xpad = np.zeros((B, CIN, HP, WP), dtype=bf16)
    xpad[:, :, PAD:PAD + H, PAD:PAD + W] = x.astype(bf16)

    # contiguous flat per-expert window blocks per (core, sample, half):
    # decimated (stride-2 gather) for even-dilation experts
    xwin_cores = []
    for c in range(NCORES):
        xw = np.zeros((NSEG, 2, CIN, WIN_FLAT), dtype=bf16)
        for s in range(NSEG):
            blocks, _ = _win_layout(pairs[s])
            for r in range(2):
                for (e, dec, p, R, Wd, off) in blocks:
                    r0 = 32 * c + 16 - p
                    c0 = 128 * r + 16 - p
                    if dec:
                        blk = xpad[s, :, r0:r0 + 2 * R:2, c0:c0 + 2 * Wd:2]
                    else:
                        blk = xpad[s, :, r0:r0 + R, c0:c0 + Wd]
                    xw[s, r, :, off:off + R * Wd] = blk.reshape(CIN, R * Wd)
        xwin_cores.append(xw)

    wt = np.empty((CIN, sum(TAPN), CE), dtype=bf16)
    for e in range(4):
        k = KS[e]
        w = ws[e].astype(np.float32)  # [CE, CIN, k, k]
        wt[:, slot_base[e]:slot_base[e] + k * k, :] = (
            w.transpose(1, 2, 3, 0).reshape(CIN, k * k, CE).astype(bf16))

    inv = (bn_scale / np.sqrt(bn_var + BN_EPS)).astype(np.float32)
    shift = (np.stack(bs) * inv + bn_bias - bn_mean * inv).astype(np.float32)
    bnp = np.stack([inv, shift], axis=1)  # [4, 2, CE]
    bnp = np.ascontiguousarray(bnp.transpose(2, 0, 1))  # [CE, 4, 2]
    return xwin_cores, wt, bnp


def run(inputs, trace=False):
    from concourse import bass_utils

    x = np.asarray(inputs["x"], dtype=np.float32)
    ws = [np.asarray(inputs[f"w{i}"], dtype=np.float32) for i in range(4)]
    bs = [np.asarray(inputs[f"b{i}"], dtype=np.float32) for i in range(4)]
    bn_scale = np.asarray(inputs["bn_scale"], dtype=np.float32)
    bn_bias = np.asarray(inputs["bn_bias"], dtype=np.float32)
    bn_mean = np.asarray(inputs["bn_mean"], dtype=np.float32)
    bn_var = np.asarray(inputs["bn_var"], dtype=np.float32)
    gate_w = np.asarray(inputs["gate_w"], dtype=np.float32)
    gate_b = np.asarray(inputs["gate_b"], dtype=np.float32)

    idx, wsel = _host_gate(x, gate_w, gate_b)
    pairs = tuple((int(idx[s, 0]), int(idx[s, 1])) for s in range(B))
    order = _seg_order(pairs)

    nc = _get_program(pairs, order)
    xwin_cores, wt, bnp = _prep_inputs(x, ws, bs, bn_scale, bn_bias,
                                       bn_mean, bn_var, pairs)
    in_maps = [{"xwin": xwin_cores[c], "wt": wt, "bnp": bnp}
               for c in range(NCORES)]
    res = bass_utils.run_bass_kernel_spmd(
        nc, in_maps, core_ids=list(range(NCORES)), trace=trace)

    # assemble: core c holds rows [16c, 16c+16) of every (sample, rank)
    E = np.empty((B, 2, CE, HO, WO), dtype=np.float32)
    for c in range(NCORES):
        o = res.results[c]["out"]  # [seg, rank, blk, rhalf, ch, row, col]
        t = (o.astype(np.float32)
             .transpose(0, 1, 4, 2, 5, 3, 6).reshape(NSEG, 2, CE, 16, 128))
        E[:, :, :, 16 * c:16 * c + 16, :] = t
    outf = np.empty((B, 2 * CE, HO, WO), dtype=np.float32)
    for s in range(B):
        outf[s, :CE] = E[s, 0] * wsel[s, 0]
        outf[s, CE:] = E[s, 1] * wsel[s, 1]
    return outf, res


def kernel(**inputs):
    outf, _ = run(inputs, trace=False)
    return outf


# revision 28
# speedup vs baseline: 1.0473x; 1.0467x over previous
"""MoE downsample kernel for 8 TRN2 NeuronCores — top-2-only compute.

The reference computes all 4 experts densely but only the host-computable
top-2 gate survives to the output, so the device computes just the 32
selected (sample, expert) convs (seed-0 demand: 1112 taps vs 2624 dense).

Work distribution keeps one compiled SPMD program with perfect balance:
every core computes output rows [16c, 16c+16) of EVERY sample (16
segments/core). Within a segment the input window is column-split across
the two PE row halves (half r covers output cols 64r..64r+63), and the
two PSUM col halves swap (expert, row-block) assignments so each of the
four 64x64 quadrant queues does exactly t_e1 + t_e2 tap-matmuls per
segment. Chunk-task = 8 rows x 64 cols = 512 px = one PSUM bank.
BN + conv-bias + GELU fuse into the ScalarE PSUM eviction. Gating and
final top-2 scale/assembly run on host.

DMA is the co-bottleneck (~47 MB moved against a ~208 GB/s packet-rate
ceiling), so: windows are staged CONTIGUOUSLY by the host and DMAed flat
(10-20 KB packets instead of ~300 B window rows), outputs are stored
bf16 (halves output bytes) on the scalar HWDGE ring while windows ride
the sync HWDGE ring, window loads are emitted two segments ahead, PSUM
banks are shared pairwise across the PSUM col halves for slack, and
light segments go first/last to shorten the DMA-bound head and the
eviction tail.
"""

import numpy as np
import ml_dtypes

KS = [3, 5, 7, 9]
DS = [1, 2, 3, 4]
PADS = [1, 4, 9, 16]       # d*(k-1)//2
TAPN = [9, 25, 49, 81]
BN_EPS = 1e-5
B, CIN, H, W = 16, 64, 256, 256
CE = 64
HO = WO = 128
NCORES = 8
PAD = 16                   # left/top pad in the padded image
HP = WP = PAD + 256 + 15   # 287
NSEG = B                   # one segment per sample per core
RB = 8                     # output rows per chunk-task block
PF = 2                     # window prefetch distance (segments)
NWARM = 48                 # dummy matmuls to pre-warm the PE HAM clock


def _blk_shape(e):
    """Per-expert window block: decimated (stride-2 gather, unit-stride
    taps) for even dilation, full-res otherwise. Returns (dec, p, R, W)."""
    p = PADS[e]
    if DS[e] % 2 == 0:
        return True, p, 16 + p, 64 + p
    return False, p, 31 + 2 * p, 127 + 2 * p


def _win_layout(pair):
    """[(e, dec, p, R, W, off)] block list + total flat length."""
    off, blocks = 0, []
    for e in pair:
        dec, p, R, Wd = _blk_shape(e)
        blocks.append((e, dec, p, R, Wd, off))
        off += R * Wd
    return blocks, off


WIN_FLAT = max(_win_layout((a, b))[1]
               for a in range(4) for b in range(4) if a != b)

_COMPILED = {}


def _tap_offsets(e):
    """Yield (local_slot, row_off, col_off) raw offsets for expert e."""
    k, d = KS[e], DS[e]
    pad = d * (k - 1) // 2
    for u in range(k):
        for v in range(k):
            yield u * k + v, d * u - pad, d * v - pad


def _seg_order(pairs):
    """Spread the lightest segments evenly (incl. first and last slots):
    light segments have the smallest windows (fast start) and shortest
    eviction tails; heavies between them give DMA prefetch headroom."""
    taps = [TAPN[a] + TAPN[b] for a, b in pairs]
    by_weight = sorted(range(NSEG), key=lambda s: (taps[s], s))
    nl = (NSEG + 2) // 3
    light_pos = sorted({round(i * (NSEG - 1) / max(nl - 1, 1))
                        for i in range(nl)})
    order = [None] * NSEG
    for pos, s in zip(light_pos, by_weight[:len(light_pos)]):
        order[pos] = s
    rest = iter(sorted(by_weight[len(light_pos):],
                       key=lambda s: (-taps[s], s)))
    for i in range(NSEG):
        if order[i] is None:
            order[i] = next(rest)
    return order


def _build_program(pairs, order, compile=True):
    import concourse.bass as bass  # noqa: F401
    import concourse.mybir as mybir
    import concourse.tile as tile
    from concourse import bacc
    from contextlib import ExitStack

    dt = mybir.dt
    nc = bacc.Bacc("TRN2", target_bir_lowering=False, debug=False,
                   num_devices=NCORES)
    xwin = nc.dram_tensor("xwin", [NSEG, 2, CIN, WIN_FLAT], dt.bfloat16,
                          kind="ExternalInput")
    wt = nc.dram_tensor("wt", [CIN, sum(TAPN), CE], dt.bfloat16,
                        kind="ExternalInput")
    bnp = nc.dram_tensor("bnp", [128, 16, 2], dt.float32,
                         kind="ExternalInput")
    # [seg, psum-col-half, row-block, col-half, ch, row, col]
    # (rank = psum-col-half XOR row-block; host remaps)
    out = nc.dram_tensor("out", [NSEG, 2, 2, 2, CE, RB, 64], dt.bfloat16,
                         kind="ExternalOutput")
    slot_base = np.cumsum([0] + [k * k for k in KS]).tolist()

    with tile.TileContext(nc) as tc:
        with ExitStack() as ctx:
            consts = ctx.enter_context(tc.tile_pool(name="consts", bufs=1))
            win_pool = ctx.enter_context(tc.tile_pool(name="win", bufs=4))
            stage_pool = ctx.enter_context(tc.tile_pool(name="st", bufs=8))
            psum_pool = ctx.enter_context(
                tc.tile_pool(name="ps", bufs=8, space="PSUM"))

            # per-expert weight tiles so early matmuls only wait on the
            # weights they actually use; first segment's experts load first
            wts = {}
            bntile = consts.tile([128, 16, 2], dt.float32)
            e_order = list(pairs[order[0]])
            e_order += [e for e in range(4) if e not in e_order]

            def load_weights(e):
                t = consts.tile([128, KS[e] * KS[e], CE], dt.bfloat16,
                                tag=f"wt{e}", name=f"wt{e}")
                for half in range(2):
                    p0 = half * 64
                    nc.sync.dma_start(
                        out=t[p0:p0 + 64, :, :],
                        in_=wt[:, slot_base[e]:slot_base[e] + KS[e] * KS[e],
                               :])
                wts[e] = t

            win_tiles = {}

            def issue_window(j):
                seg = order[j]
                _, flat_len = _win_layout(pairs[seg])
                win = win_pool.tile([128, WIN_FLAT], dt.bfloat16, name="win")
                for r in range(2):
                    nc.sync.dma_start(
                        out=win[64 * r:64 * r + 64, 0:flat_len],
                        in_=xwin[seg, r, :, 0:flat_len])
                win_tiles[j] = win

            load_weights(e_order[0])
            load_weights(e_order[1])
            nc.sync.dma_start(out=bntile[0:128, :, :], in_=bnp.ap())
            issue_window(0)
            for e in e_order[2:]:
                load_weights(e)
            for j in range(1, 1 + PF):
                issue_window(j)

            # keep the PE HAM clock warm through the startup DMA wait:
            # zero matmuls into the first segment's bank; the real first
            # tap has start=True so the garbage is overwritten
            scratch = consts.tile([128, 512], dt.bfloat16)
            nc.vector.memset(scratch, 0.0)

            for j in range(NSEG):
                seg = order[j]
                e1, e2 = pairs[seg]
                if j + PF + 1 < NSEG:
                    issue_window(j + PF + 1)
                win = win_tiles.pop(j)
                # per-expert [128, R, W] views of the flat-packed blocks
                views = {}
                for (e, dec, pe, R, Wd, off) in _win_layout((e1, e2))[0]:
                    views[e] = (win[:, off:off + R * Wd].rearrange(
                        "p (r w) -> p r w", w=Wd), dec, pe)
                # PSUM banks shared across the two col-halves per
                # (phase, row-half): 4 live banks per segment
                ps_seg = {(ph, r): psum_pool.tile([128, 512], dt.float32,
                                                  name="psb")
                          for ph in range(2) for r in range(2)}
                if j == 0:
                    for _ in range(NWARM):
                        nc.tensor.matmul(ps_seg[(0, 0)][0:64, :],
                                         scratch[0:64, 0:64],
                                         scratch[0:64, :], start=True,
                                         stop=True, tile_position=(0, 0))

                def mm_events(e, blk, r, c, ph, views=views,
                              ps_seg=ps_seg):
                    ps = ps_seg[(ph, r)]
                    q0 = c * 64
                    p0 = r * 64
                    taps = list(_tap_offsets(e))
                    wte = wts[e]
                    view, dec, pe = views[e]
                    for t, (slot, ro, co) in enumerate(taps):
                        if dec:
                            r_lo = 8 * blk + ro // 2 + pe // 2
                            c_lo = co // 2 + pe // 2
                            rhs = view[p0:p0 + 64, r_lo:r_lo + 8,
                                       c_lo:c_lo + 64]
                        else:
                            r_lo = 16 * blk + ro + pe
                            c_lo = co + pe
                            rhs = view[p0:p0 + 64, r_lo:r_lo + 15:2,
                                       c_lo:c_lo + 127:2]
                        lhsT = wte[p0:p0 + 64, slot, :]
                        psv = ps[q0:q0 + 64, :]
                        first = t == 0
                        last = t == len(taps) - 1

                        def mm(rhs=rhs, lhsT=lhsT, psv=psv, first=first,
                               last=last, p0=p0, q0=q0):
                            nc.tensor.matmul(psv, lhsT, rhs, start=first,
                                             stop=last,
                                             tile_position=(p0, q0))
                        yield mm

                def bank_evict(ph, r, pa, pb, seg=seg, ps_seg=ps_seg):
                    # one ACTIVATE + one store for the whole 128-partition
                    # bank: partitions 0-63 = expert pa (psum col-half 0),
                    # 64-127 = pb; combined BN vector from the pair table
                    def evict():
                        st = stage_pool.tile([128, RB, 64], dt.bfloat16,
                                             name="st")
                        combo = 4 * pa + pb
                        nc.scalar.activation(
                            st[0:128, :, :],
                            ps_seg[(ph, r)].rearrange(
                                "p (a b) -> p a b", a=RB),
                            mybir.ActivationFunctionType.Gelu,
                            scale=bntile[0:128, combo, 0:1],
                            bias=bntile[0:128, combo, 1:2])
                        nc.scalar.dma_start(
                            out=out[seg, :, ph, r, :, :, :],
                            in_=st[0:128, :, :])
                    return evict

                def qgen(r, c, e1=e1, e2=e2):
                    ea, eb = (e1, e2) if c == 0 else (e2, e1)
                    for ph, e_mine in ((0, ea), (1, eb)):
                        yield from mm_events(e_mine, ph, r, c, ph)
                        e_other = e2 if e_mine == e1 else e1
                        if TAPN[e_mine] > TAPN[e_other]:
                            pa = e1 if ph == 0 else e2
                            pb = e2 if ph == 0 else e1
                            yield bank_evict(ph, r, pa, pb)

                queues = [qgen(r, c) for r in range(2) for c in range(2)]
                live = list(queues)
                while live:
                    nxt = []
                    for q in live:
                        ev = next(q, None)
                        if ev is None:
                            continue
                        ev()
                        nxt.append(q)
                    live = nxt

    if compile:
        nc.compile()
    return nc


def _get_program(pairs, order):
    key = (pairs, tuple(order))
    if key not in _COMPILED:
        _COMPILED[key] = _build_program(pairs, order)
    return _COMPILED[key]


def _host_gate(x, gate_w, gate_b):
    """Replicate reference gating in numpy (f64 pooling for robustness)."""
    pooled = x.astype(np.float64).mean(axis=(2, 3)).astype(np.float32)
    logits = pooled @ gate_w.T.astype(np.float32) + gate_b
    z = logits - logits.max(axis=1, keepdims=True)
    ez = np.exp(z.astype(np.float32))
    gates = ez / ez.sum(axis=1, keepdims=True)
    idx = np.argsort(-gates, axis=1, kind="stable")[:, :2]
    wsel = np.take_along_axis(gates, idx, axis=1)
    wsel = wsel / (wsel.sum(axis=1, keepdims=True) + 1e-8)
    return idx, wsel.astype(np.float32)


def _prep_inputs(x, ws, bs, bn_scale, bn_bias, bn_mean, bn_var, pairs):
    bf16 = ml_dtypes.bfloat16
    slot_base = np.cumsum([0] + [k * k for k in KS]).tolist()
    xpad = np.zeros((B, CIN, HP, WP), dtype=bf16)
    xpad[:, :, PAD:PAD + H, PAD:PAD + W] = x.astype(bf16)

    # contiguous flat per-expert window blocks per (core, sample, half):
    # decimated (stride-2 gather) for even-dilation experts
    xwin_cores = []
    for c in range(NCORES):
        xw = np.zeros((NSEG, 2, CIN, WIN_FLAT), dtype=bf16)
        for s in range(NSEG):
            blocks, _ = _win_layout(pairs[s])
            for r in range(2):
                for (e, dec, p, R, Wd, off) in blocks:
                    r0 = 32 * c + 16 - p
                    c0 = 128 * r + 16 - p
                    if dec:
                        blk = xpad[s, :, r0:r0 + 2 * R:2, c0:c0 + 2 * Wd:2]
                    else:
                        blk = xpad[s, :, r0:r0 + R, c0:c0 + Wd]
                    xw[s, r, :, off:off + R * Wd] = blk.reshape(CIN, R * Wd)
        xwin_cores.append(xw)

    wt = np.empty((CIN, sum(TAPN), CE), dtype=bf16)
    for e in range(4):
        k = KS[e]
        w = ws[e].astype(np.float32)  # [CE, CIN, k, k]
        wt[:, slot_base[e]:slot_base[e] + k * k, :] = (
            w.transpose(1, 2, 3, 0).reshape(CIN, k * k, CE).astype(bf16))

    inv = (bn_scale / np.sqrt(bn_var + BN_EPS)).astype(np.float32)
    shift = (np.stack(bs) * inv + bn_bias - bn_mean * inv).astype(np.float32)
    # combined per-ordered-pair BN vectors: partitions 0-63 = expert a
    # (psum col-half 0), 64-127 = expert b
    bnp = np.zeros((128, 16, 2), np.float32)
    for a in range(4):
        for b in range(4):
            idx = 4 * a + b
            bnp[0:CE, idx, 0] = inv[a]
            bnp[0:CE, idx, 1] = shift[a]
            bnp[CE:, idx, 0] = inv[b]
            bnp[CE:, idx, 1] = shift[b]
    return xwin_cores, wt, bnp


def run(inputs, trace=False):
    from concourse import bass_utils

    x = np.asarray(inputs["x"], dtype=np.float32)
    ws = [np.asarray(inputs[f"w{i}"], dtype=np.float32) for i in range(4)]
    bs = [np.asarray(inputs[f"b{i}"], dtype=np.float32) for i in range(4)]
    bn_scale = np.asarray(inputs["bn_scale"], dtype=np.float32)
    bn_bias = np.asarray(inputs["bn_bias"], dtype=np.float32)
    bn_mean = np.asarray(inputs["bn_mean"], dtype=np.float32)
    bn_var = np.asarray(inputs["bn_var"], dtype=np.float32)
    gate_w = np.asarray(inputs["gate_w"], dtype=np.float32)
    gate_b = np.asarray(inputs["gate_b"], dtype=np.float32)

    idx, wsel = _host_gate(x, gate_w, gate_b)
    pairs = tuple((int(idx[s, 0]), int(idx[s, 1])) for s in range(B))
    order = _seg_order(pairs)

    nc = _get_program(pairs, order)
    xwin_cores, wt, bnp = _prep_inputs(x, ws, bs, bn_scale, bn_bias,
                                       bn_mean, bn_var, pairs)
    in_maps = [{"xwin": xwin_cores[c], "wt": wt, "bnp": bnp}
               for c in range(NCORES)]
    res = bass_utils.run_bass_kernel_spmd(
        nc, in_maps, core_ids=list(range(NCORES)), trace=trace)

    # assemble: core c holds rows [16c, 16c+16) of every (sample, rank)
    E = np.empty((B, 2, CE, HO, WO), dtype=np.float32)
    for c in range(NCORES):
        o = res.results[c]["out"]  # [seg, chalf, blk, rhalf, ch, row, col]
        orank = np.empty_like(o)   # rank = chalf XOR blk
        orank[:, 0, 0] = o[:, 0, 0]
        orank[:, 0, 1] = o[:, 1, 1]
        orank[:, 1, 0] = o[:, 1, 0]
        orank[:, 1, 1] = o[:, 0, 1]
        t = (orank.astype(np.float32)
             .transpose(0, 1, 4, 2, 5, 3, 6).reshape(NSEG, 2, CE, 16, 128))
        E[:, :, :, 16 * c:16 * c + 16, :] = t
    outf = np.empty((B, 2 * CE, HO, WO), dtype=np.float32)
    for s in range(B):
        outf[s, :CE] = E[s, 0] * wsel[s, 0]
        outf[s, CE:] = E[s, 1] * wsel[s, 1]
    return outf, res


def kernel(**inputs):
    outf, _ = run(inputs, trace=False)
    return outf
